# revision 43
# baseline (speedup 1.0000x reference)
"""Trainium2 Bass kernel for a dense transformer block.

Strategy: data-parallel over tokens (8 shards of 512 tokens; 4 shards
per batch element, one per core).  Each core computes the full block
for its own tokens.  Attention needs K/V for the whole 2048-token
sequence of the core's batch element.

v2 changes vs the 505us baseline (trace-driven):
  - The K/V exchange is split into EIGHT per-head-pair AllGathers
    (0.25 MiB each) triggered as soon as that pair's K and V tiles are
    stored, so attention pair p starts as soon as gather p lands
    (~40us) instead of waiting for two serialized 50us collectives
    (165us).
  - LayerNorm1 never blocks the PE: Q/K/V matmuls run on RAW x; the
    mean correction is folded in as one extra K=1 matmul per psum
    chain (psum -= colsum(W) x mean), and the 1/std scale is folded
    into the psum->SBUF copies (DVE mul for Q/K, per-partition
    activation scale for V).
  - reciprocal_approx_fast replaces the 3.3us 1-lane reciprocals;
    LN broadcasts ride the PE (ones-matmul) instead of gpsimd.
  - Attention normalize multiplies straight out of PSUM (no oe copy).

Layout: activations live in SBUF as [feature(partition), token(free)]
tiles; weights are pre-transposed/pre-tiled on the host so every
matmul is lhsT[K=128, M=128] x rhs[K=128, N=512].  The residual spine
is fp32; everything wide (Q/K/V, softmax probabilities, weights, MLP
hidden) is fp16.  V is produced directly in [token, feature] layout
(x as the stationary matmul operand) so P@V needs no transposes, and
a ones-column appended to V yields the softmax denominators in the
same accumulation (M=65).
"""

import contextlib

import numpy as np

import concourse.bass as bass  # noqa: F401
import concourse.mybir as mybir
import concourse.tile as tile
from concourse import bacc
from concourse import bass_utils

F32 = mybir.dt.float32
F32R = mybir.dt.float32r
F16 = mybir.dt.float16
F8 = mybir.dt.float8e4
I32 = mybir.dt.int32
AF = mybir.ActivationFunctionType
import numpy as _np
EXP_A = 0.125 * float(_np.log2(_np.e)) * (1 << 23)
EXP_B = float((127 << 23) - 486411)

DIM = 1024
HEADS = 16
HD = 64
HIDDEN = 4096
B = 2
L = 2048
N_CORES = 8
TOK = 512           # tokens per core
DT = DIM // 128     # 8 feature tiles
HT = HIDDEN // 128  # 32 hidden tiles
NPAIR = HEADS // 2  # 8 head pairs (128 features each)
RANKS = 4           # cores per batch group
GROUPS = [[0, 1, 2, 3], [4, 5, 6, 7]]


def _r(ap):
    return ap.bitcast(F32R)


def build():
    nc = bacc.Bacc("TRN2", target_bir_lowering=False, debug=False,
                   num_devices=N_CORES)

    xh = nc.dram_tensor("xh", [DIM, TOK], F16, kind="ExternalInput").ap()
    xT = nc.dram_tensor("xT", [DIM, TOK], F32, kind="ExternalInput").ap()
    # lhsT-tiled weights: [m_tiles, 128(k_inner), k_tiles, 128(m_inner)]
    wqk = nc.dram_tensor("wqk", [16, 128, DT, 128], F16, kind="ExternalInput").ap()
    wv = nc.dram_tensor("wv", [DT, 128, DIM], F16, kind="ExternalInput").ap()
    nws = nc.dram_tensor("nws", [1, 16, 128], F16, kind="ExternalInput").ap()
    wvs = nc.dram_tensor("wvs", [1, 2, TOK], F16, kind="ExternalInput").ap()
    wproj = nc.dram_tensor("wproj", [DT, 128, DT, 128], F16, kind="ExternalInput").ap()
    w1 = nc.dram_tensor("w1", [HT, 128, DT, 128], F16, kind="ExternalInput").ap()
    w2 = nc.dram_tensor("w2", [DT, 128, HT, 128], F16, kind="ExternalInput").ap()
    yT = nc.dram_tensor("yT", [DIM, TOK], F32, kind="ExternalOutput").ap()

    with tile.TileContext(nc) as tc:
        with contextlib.ExitStack() as ctx:
            # ---- long-lived pools -------------------------------------
            const = ctx.enter_context(tc.tile_pool(name="const", bufs=1))
            rows = ctx.enter_context(tc.tile_pool(name="rows", bufs=3))
            rows1 = ctx.enter_context(tc.tile_pool(name="rows1", bufs=1))
            xhp = ctx.enter_context(tc.tile_pool(name="xhp", bufs=8))
            x32p = ctx.enter_context(tc.tile_pool(name="x32p", bufs=2))
            qp_pool = ctx.enter_context(tc.tile_pool(name="qp", bufs=8))
            atp = ctx.enter_context(tc.tile_pool(name="atp", bufs=8))
            Xp = ctx.enter_context(tc.tile_pool(name="Xp", bufs=8))
            ynorm = ctx.enter_context(tc.tile_pool(name="ynorm", bufs=8))
            yop = ctx.enter_context(tc.tile_pool(name="yop", bufs=1))
            dram = ctx.enter_context(tc.tile_pool(name="dram", bufs=1, space="DRAM"))
            # attention SBUF pools; allocate (pin) every ring buffer NOW,
            # before the QKV-phase pools claim space, so the attention
            # tiles never alias QKV tiles -> no false serialization at
            # the QKV->attention handoff via address reuse.
            kp_pool = ctx.enter_context(tc.tile_pool(name="kp", bufs=6))
            kp8_pool = ctx.enter_context(tc.tile_pool(name="kp8", bufs=4))
            va_pool = ctx.enter_context(tc.tile_pool(name="va", bufs=6))
            va8_pool = ctx.enter_context(tc.tile_pool(name="va8", bufs=6))
            ex_pool = ctx.enter_context(tc.tile_pool(name="ex", bufs=4))
            bcr_p = ctx.enter_context(tc.tile_pool(name="bcr", bufs=2))
            for i in range(6):
                t = kp_pool.tile([128, TOK], F16, tag="kp", name=f"kpd{i}")
                nc.vector.memset(t[0:1, 0:1], 0.0)
                if i < 4:
                    t = kp8_pool.tile([128, TOK], F8, tag="kp8", name=f"kp8d{i}")
                    nc.vector.memset(t[0:1, 0:1], 0.0)
            for i in range(6):
                t = va_pool.tile([128, 4, 2, HD + 1], F16, tag="va",
                                 name=f"vad{i}")
                nc.vector.memset(t[0:1, 0:1, 0:1, 0:1], 0.0)
                t = va8_pool.tile([128, 4, 2, HD], F8, tag="va8",
                                  name=f"va8d{i}")
                nc.vector.memset(t[0:1, 0:1, 0:1, 0:1], 0.0)
            for i in range(4):
                t = ex_pool.tile([128, 2, TOK], F16, tag="ex", name=f"exd{i}")
                nc.vector.memset(t[0:1, 0:1, 0:1], 0.0)
            for i in range(2):
                t = bcr_p.tile([HD, TOK], F32, tag="bcr", name=f"bcrd{i}")
                nc.vector.memset(t[0:1, 0:1], 0.0)


            ones16 = const.tile([128, 1], F16)
            nc.vector.memset(ones16[:], 1.0)
            ones32 = const.tile([128, 1], F32)
            nc.vector.memset(ones32[:], 1.0)
            onesrow = const.tile([1, 128], F32)
            nc.vector.memset(onesrow[:], 1.0)
            nws_sb = const.tile([1, 16, 128], F16)
            nc.scalar.dma_start(out=nws_sb[:], in_=nws)
            wvs_sb = const.tile([1, 2, TOK], F16)
            nc.scalar.dma_start(out=wvs_sb[:], in_=wvs)
            r_col4 = const.tile([128, 4], F32)

            warm_in = dram.tile([1, 16], F16)
            warm_out = dram.tile([RANKS, 1, 16], F16)
            kv_in = [dram.tile([2, 128, TOK], F8, name=f"kvi{p}")
                     for p in range(NPAIR)]
            kv_out = [dram.tile([RANKS, 2, 128, TOK], F8, name=f"kvo{p}")
                      for p in range(NPAIR)]

            # Warm up the collective subsystem (init barrier + ncfw)
            # immediately so the real gathers below do not pay the
            # first-collective penalty.
            wz = rows.tile([1, 16], F16, tag="wz")
            nc.vector.memset(wz[:], 0.0)
            nc.scalar.dma_start(out=warm_in[0], in_=wz[:])
            nc.gpsimd.collective_compute(
                "AllGather", mybir.AluOpType.bypass,
                replica_groups=GROUPS,
                ins=[warm_in.opt()], outs=[warm_out.opt()])

            # ---- load x (fp16 for matmuls; fp32 later for residual) ----
            xh_tiles = []
            for dc in range(DT):
                t = xhp.tile([128, TOK], F16, tag="xh")
                nc.sync.dma_start(out=t[:], in_=xh[dc * 128:(dc + 1) * 128, :])
                xh_tiles.append(t)

            with contextlib.ExitStack() as actx:
                sqp = actx.enter_context(tc.tile_pool(name="sqp", bufs=2))
                wqk_pool = actx.enter_context(tc.tile_pool(name="wqkp", bufs=16))
                wv_pool = actx.enter_context(tc.tile_pool(name="wvp", bufs=2))
                kvtmp = actx.enter_context(tc.tile_pool(name="kvtmp", bufs=4))
                psQ = actx.enter_context(tc.tile_pool(name="psQ", bufs=2, space="PSUM"))
                lnps = actx.enter_context(tc.tile_pool(name="lnps", bufs=2, space="PSUM"))
                rbcps = actx.enter_context(tc.tile_pool(name="rbcps", bufs=1, space="PSUM"))
                ps1 = actx.enter_context(tc.tile_pool(name="ps1", bufs=2, space="PSUM"))

                # ---- LN1 stats on fp16 raw x (rides along) ------------
                sum_ps = lnps.tile([1, TOK], F32, tag="lnps", name="sum")[:]
                sq_ps = lnps.tile([1, TOK], F32, tag="lnps", name="sq")[:]
                sq_tiles = []
                for dc in range(DT):
                    sq = sqp.tile([128, TOK], F16, tag="sq")
                    nc.vector.tensor_mul(sq[:], xh_tiles[dc][:], xh_tiles[dc][:])
                    sq_tiles.append(sq)
                for dc in range(DT):
                    nc.tensor.matmul(sum_ps, ones16[:], xh_tiles[dc][:],
                                     start=(dc == 0), stop=(dc == DT - 1),
                                     skip_group_check=True)
                for dc in range(DT):
                    nc.tensor.matmul(sq_ps, ones16[:], sq_tiles[dc][:],
                                     start=(dc == 0), stop=(dc == DT - 1),
                                     skip_group_check=True)

                m16 = rows.tile([1, TOK], F16, tag="r16")
                negm16 = rows.tile([1, TOK], F16, tag="r16")
                m32 = rows.tile([1, TOK], F32, tag="r32")
                ex2 = rows.tile([1, TOK], F32, tag="r32")
                var = rows.tile([1, TOK], F32, tag="r32")
                std = rows.tile([1, TOK], F32, tag="r32")
                r32 = rows.tile([1, TOK], F32, tag="r32")
                nc.vector.tensor_scalar_mul(m16[:], sum_ps, 1.0 / DIM)
                nc.vector.tensor_scalar_mul(negm16[:], sum_ps, -1.0 / DIM)
                nc.vector.tensor_scalar_mul(m32[:], sum_ps, 1.0 / DIM)
                nc.vector.tensor_scalar_mul(ex2[:], sq_ps, 1.0 / DIM)
                nc.vector.tensor_mul(var[:], m32[:], m32[:])
                nc.vector.tensor_sub(var[:], ex2[:], var[:])
                nc.scalar.activation(std[:], var[:], AF.Sqrt)
                nc.vector.reciprocal_approx_fast(r32[:], std[:])
                # broadcast r over partitions via the PE; column-chunked r
                # for the V-side per-partition scale via a tiny DMA gather
                rbc = rbcps.tile([128, TOK], F32, tag="rbc")
                nc.tensor.matmul(rbc[:], onesrow[:], r32[:],
                                 start=True, stop=True)
                rbc_sb = xhp.tile([128, TOK], F32, tag="rbcsb", bufs=1)
                nc.vector.tensor_copy(rbc_sb[:], rbc[:])
                r_scr = dram.tile([1, TOK], F32, name="rscr")
                nc.sync.dma_start(out=r_scr[:], in_=r32[:])
                nc.sync.dma_start(
                    out=r_col4[:],
                    in_=r_scr[0:1, :].rearrange("o (c p) -> (o p) c", p=128))

                def qk_chain(et, dest_pool, dest_tag, dest_name,
                             dest_dtype=F16):
                    """One Q or K etile: 8 raw MMs + mean-correction MM,
                    then scale-by-r copy to SBUF."""
                    wt = wqk_tiles[et]
                    ps_pool = psQ if et < 8 else ps1
                    ps = ps_pool.tile([128, TOK], F32, tag="mm", name=f"qk{et}")
                    for dc in range(DT):
                        nc.tensor.matmul(ps[:], wt[:, dc, :],
                                         xh_tiles[dc][:],
                                         start=(dc == 0), stop=False)
                    nc.tensor.matmul(ps[:], nws_sb[:, et, :], m16[:],
                                     start=False, stop=True)
                    dest = dest_pool.tile([128, TOK], dest_dtype, tag=dest_tag,
                                          name=dest_name)
                    nc.vector.tensor_mul(dest[:], ps[:], rbc_sb[:])
                    return dest

                # contiguous per-etile weight loads; K etiles (8..15) first
                wqk_tiles = [None] * 16
                for et in list(range(8, 16)) + list(range(8)):
                    wt = wqk_pool.tile([128, DT, 128], F16, tag="wqk",
                                       name=f"wqk{et}")
                    nc.sync.dma_start(out=wt[:], in_=wqk[et])
                    wqk_tiles[et] = wt

                def v_block(nh):
                    """V for feature half nh: 4 token-chunk chains, each
                    [128 tok, 512 feat]; store per-pair slices."""
                    wvt = wv_pool.tile([128, DT, TOK], F16, tag="wv",
                                       name=f"wv{nh}")
                    nc.sync.dma_start(
                        out=wvt[:],
                        in_=wv[:, :, nh * TOK:(nh + 1) * TOK].rearrange(
                            "d k f -> k d f"))
                    for tt in range(4):
                        ps = ps1.tile([128, TOK], F32, tag="mm",
                                      name=f"v{nh}_{tt}")
                        for dc in range(DT):
                            nc.tensor.matmul(
                                ps[:], xh_tiles[dc][:, tt * 128:(tt + 1) * 128],
                                wvt[:, dc, :],
                                start=(dc == 0), stop=False)
                        nc.tensor.matmul(
                            ps[:], negm16[:, tt * 128:(tt + 1) * 128],
                            wvs_sb[:, nh, :], start=False, stop=True)
                        vt = kvtmp.tile([128, TOK], F8, tag="kv",
                                        name=f"vt{nh}_{tt}")
                        nc.scalar.activation(vt[:], ps[:], AF.Copy,
                                             scale=r_col4[:, tt:tt + 1])
                        for j in range(4):
                            p = nh * 4 + j
                            nc.scalar.dma_start(
                                out=kv_in[p][1, :, tt * 128:(tt + 1) * 128],
                                in_=vt[:, j * 128:(j + 1) * 128])

                def k_chain(p):
                    kt = qk_chain(8 + p, kvtmp, "kv", f"kt{p}", F8)
                    nc.scalar.dma_start(out=kv_in[p][0], in_=kt[:])

                def gather(p):
                    nc.gpsimd.collective_compute(
                        "AllGather", mybir.AluOpType.bypass,
                        replica_groups=GROUPS,
                        ins=[kv_in[p].opt()], outs=[kv_out[p].opt()])

                q_tiles = [None] * NPAIR

                for p in range(4):
                    k_chain(p)
                v_block(0)
                for p in range(4):
                    gather(p)
                for p in range(NPAIR):
                    q_tiles[p] = qk_chain(p, qp_pool, "q", f"q{p}")
                for p in range(4, 8):
                    k_chain(p)
                v_block(1)
                for p in range(4, 8):
                    gather(p)

            # ---- attention + weight prefetch --------------------------
            attn_tiles = []
            wp_tiles = []
            w1_tiles = []
            with tc.tile_pool(name="wproj", bufs=2) as wp_pool, \
                 tc.tile_pool(name="w1", bufs=4) as w1_pool, \
                 contextlib.ExitStack() as attps:
                pss = attps.enter_context(
                    tc.tile_pool(name="pss", bufs=2, space="PSUM"))
                pso = attps.enter_context(
                    tc.tile_pool(name="pso", bufs=4, space="PSUM"))

                def load_w1(g):
                    wt = w1_pool.tile([128, 4, DT, 128], F16, tag="w1",
                                      name=f"w1_{g}")
                    nc.sync.dma_start(
                        out=wt[:],
                        in_=w1[g * 4:(g + 1) * 4].rearrange("e k d m -> k e d m"))
                    w1_tiles.append(wt)

                def prefetch(p):
                    # trickle the proj/fc1 weights in between pairs; only
                    # into FRESH ring slots (a recycled slot's DMA would
                    # park in the sync queue waiting on MLP-time readers
                    # and stall the attention loads queued behind it)
                    if p < 2:
                        wt = wp_pool.tile([128, 4, DT, 128], F16, tag="wp",
                                          name=f"wp{p}")
                        nc.sync.dma_start(
                            out=wt[:],
                            in_=wproj[p * 4:(p + 1) * 4].rearrange(
                                "e k d m -> k e d m"))
                        wp_tiles.append(wt)
                    if p < 4:
                        load_w1(p)

                for p in range(NPAIR):
                    qp = q_tiles[p]
                    kps, vas = [], []
                    for r_i in range(RANKS):
                        kp8 = kp8_pool.tile([128, TOK], F8, tag="kp8",
                                            name=f"kp8{p}_{r_i}")
                        nc.sync.dma_start(out=kp8[:],
                                          in_=kv_out[p][r_i, 0])
                        kp = kp_pool.tile([128, TOK], F16, tag="kp",
                                          name=f"kp{p}_{r_i}")
                        nc.vector.tensor_copy(kp[:], kp8[:])
                        kps.append(kp)
                    for r_i in range(RANKS):
                        va8 = va8_pool.tile([128, 4, 2, HD], F8, tag="va8",
                                            name=f"va8{p}_{r_i}")
                        nc.sync.dma_start(
                            out=va8[:],
                            in_=kv_out[p][r_i, 1].rearrange(
                                "t (c h d) -> t c h d", c=4, d=HD))
                        va = va_pool.tile([128, 4, 2, HD + 1], F16,
                                          tag="va", name=f"va{p}_{r_i}")
                        nc.gpsimd.tensor_copy(va[:, :, :, 0:HD], va8[:])
                        nc.vector.memset(va[:, :, :, HD:HD + 1], 1.0)
                        vas.append(va)
                    o0 = pso.tile([HD + 1, TOK], F32, tag="pso",
                                  name=f"o0_{p}")
                    o1 = pso.tile([HD + 1, TOK], F32, tag="pso",
                                  name=f"o1_{p}")
                    for kt in range(16):
                        r_i, cc = kt // 4, kt % 4
                        kp, va = kps[r_i], vas[r_i]
                        ss = pss.tile([128, 2, TOK], F32, tag="pss")
                        ex = ex_pool.tile([128, 2, TOK], F16, tag="ex")
                        nc.tensor.matmul(
                            ss[:, 0, :],
                            kp[0:HD, cc * 128:(cc + 1) * 128],
                            qp[0:HD, :], start=True, stop=True)
                        nc.tensor.matmul(
                            ss[:, 1, :],
                            kp[HD:128, cc * 128:(cc + 1) * 128],
                            qp[HD:128, :], start=True, stop=True)
                        if kt in (4, 9, 14):
                            # Schraudolph exp2 on the DVE (error ~1.8%
                            # rms on probabilities; net effect ~1e-4 on
                            # the block output) to offload the scalar
                            # engine, which paces the attention phase
                            nc.vector.tensor_scalar(
                                ss[:].bitcast(I32), ss[:], EXP_A, EXP_B,
                                mybir.AluOpType.mult, mybir.AluOpType.add)
                            nc.vector.tensor_copy(ex[:], ss[:])
                        else:
                            nc.scalar.activation(ex[:], ss[:], AF.Exp,
                                                 scale=float(HD) ** -0.5)
                        nc.tensor.matmul(o0[:], va[:, cc, 0, :],
                                         ex[:, 0, :],
                                         start=(kt == 0), stop=(kt == 15))
                        nc.tensor.matmul(o1[:], va[:, cc, 1, :],
                                         ex[:, 1, :],
                                         start=(kt == 0), stop=(kt == 15))
                    at = atp.tile([128, TOK], F16, tag="at", name=f"at{p}")
                    for h_i, o in ((0, o0), (1, o1)):
                        oe = ex_pool.tile([HD + 1, TOK], F32, tag="oe",
                                          bufs=2)
                        nc.vector.tensor_copy(oe[:], o[:])
                        # reciprocal_approx_* mishandles base_partition!=0
                        # inputs -> bounce the denominator row to part 0
                        den = rows1.tile([1, TOK], F32, tag="den")
                        nc.vector.tensor_copy(den[:], oe[HD:HD + 1, :])
                        rc = rows.tile([1, TOK], F32, tag="rc")
                        nc.vector.reciprocal_approx_fast(rc[:], den[:])
                        bcr = bcr_p.tile([HD, TOK], F32, tag="bcr")
                        nc.gpsimd.partition_broadcast(bcr[:], rc[:])
                        nc.vector.tensor_mul(at[h_i * HD:(h_i + 1) * HD, :],
                                             oe[0:HD, :], bcr[:])
                    attn_tiles.append(at)
                    prefetch(p)

                attps.close()
                # ---- proj + residual, LN2 (stats interleaved) ---------
                X_tiles = []
                with tc.tile_pool(name="tmp", bufs=4) as tmp, \
                     tc.tile_pool(name="ps3", bufs=4, space="PSUM") as ps3, \
                     tc.tile_pool(name="lnps2", bufs=2, space="PSUM") as lnps2, \
                     tc.tile_pool(name="bc2", bufs=2, space="PSUM") as bc2:
                    sum2 = lnps2.tile([1, TOK], F32, tag="lnps")
                    sq2 = lnps2.tile([1, TOK], F32, tag="lnps")
                    x_tiles = []
                    for dc in range(DT):
                        t = x32p.tile([128, TOK], F32, tag="x",
                                      name=f"x{dc}")
                        nc.scalar.dma_start(
                            out=_r(t[:]),
                            in_=_r(xT[dc * 128:(dc + 1) * 128, :]))
                        x_tiles.append(t)
                    for et in range(DT):
                        wt = wp_tiles[et // 4]
                        ps = ps3.tile([128, TOK], F32, tag="mm")
                        for dc in range(DT):
                            nc.tensor.matmul(ps[:], wt[:, et % 4, dc, :],
                                             attn_tiles[dc][:],
                                             start=(dc == 0),
                                             stop=(dc == DT - 1))
                        xt = Xp.tile([128, TOK], F32, tag="X",
                                     name=f"X{et}")
                        nc.vector.tensor_add(_r(xt[:]), ps[:],
                                             x_tiles[et][:])
                        X_tiles.append(xt)
                        sq = tmp.tile([128, TOK], F32, tag="lntmp", bufs=3)
                        nc.vector.tensor_mul(_r(sq[:]), xt[:], xt[:])
                        nc.tensor.matmul(sum2[:], _r(ones32[:]), _r(xt[:]),
                                         start=(et == 0), stop=(et == DT - 1))
                        nc.tensor.matmul(sq2[:], _r(ones32[:]), _r(sq[:]),
                                         start=(et == 0), stop=(et == DT - 1))

                    m2 = rows.tile([1, TOK], F32, tag="r32b")
                    e2 = rows.tile([1, TOK], F32, tag="r32b")
                    v2 = rows.tile([1, TOK], F32, tag="r32b")
                    s2 = rows.tile([1, TOK], F32, tag="r32b")
                    r2 = rows.tile([1, TOK], F32, tag="r32b")
                    nc.vector.tensor_scalar_mul(m2[:], sum2[:], 1.0 / DIM)
                    mbc = bc2.tile([128, TOK], F32, tag="bc")
                    nc.tensor.matmul(mbc[:], onesrow[:], m2[:],
                                     start=True, stop=True)
                    nc.vector.tensor_scalar_mul(e2[:], sq2[:], 1.0 / DIM)
                    nc.vector.tensor_mul(v2[:], m2[:], m2[:])
                    nc.vector.tensor_sub(v2[:], e2[:], v2[:])
                    nc.scalar.activation(s2[:], v2[:], AF.Sqrt)
                    nc.vector.reciprocal_approx_fast(r2[:], s2[:])
                    rbc2 = bc2.tile([128, TOK], F32, tag="bc")
                    nc.tensor.matmul(rbc2[:], onesrow[:], r2[:],
                                     start=True, stop=True)
                    Y_tiles = []
                    for dc in range(DT):
                        tm = tmp.tile([128, TOK], F32, tag="lnap")
                        nc.vector.tensor_sub(tm[:], X_tiles[dc][:], mbc[:])
                        y = ynorm.tile([128, TOK], F16, tag="y",
                                       name=f"y{dc}")
                        nc.vector.tensor_mul(y[:], tm[:], rbc2[:])
                        Y_tiles.append(y)

                # ---- fc1 + gelu, fc2 + residual -----------------------
                with tc.tile_pool(name="hp", bufs=32) as hp, \
                     tc.tile_pool(name="ps4", bufs=4, space="PSUM") as ps4:
                    for g in range(4, 8):
                        load_w1(g)
                    h_tiles = []
                    for ht in range(HT):
                        wt = w1_tiles[ht // 4]
                        ps = ps4.tile([128, TOK], F32, tag="mm")
                        for dc in range(DT):
                            nc.tensor.matmul(ps[:], wt[:, ht % 4, dc, :],
                                             Y_tiles[dc][:],
                                             start=(dc == 0),
                                             stop=(dc == DT - 1))
                        h = hp.tile([128, TOK], F16, tag="h")
                        nc.scalar.activation(h[:], ps[:], AF.Gelu)
                        h_tiles.append(h)

                    with tc.tile_pool(name="w2", bufs=2) as w2_pool:
                        for et in range(DT):
                            wts = []
                            for hh in range(2):
                                wt = w2_pool.tile([128, HT // 2, 128], F16,
                                                  tag="w2")
                                nc.sync.dma_start(
                                    out=wt[:],
                                    in_=w2[et][:, hh * 16:(hh + 1) * 16, :])
                                wts.append(wt)
                            ps = ps4.tile([128, TOK], F32, tag="mm")
                            for hc in range(HT):
                                nc.tensor.matmul(ps[:], wts[hc // 16][:, hc % 16, :],
                                                 h_tiles[hc][:],
                                                 start=(hc == 0),
                                                 stop=(hc == HT - 1))
                            ot = yop.tile([128, TOK], F32, tag="yo")
                            nc.vector.tensor_add(ot[:], ps[:],
                                                 X_tiles[et][:])
                            nc.sync.dma_start(
                                out=yT[et * 128:(et + 1) * 128, :],
                                in_=ot[:])

    nc.compile()
    return nc


def _tile_lhsT(wT, kt, mt, dtype=np.float16):
    """[Ktot, Mtot] -> [mt, 128, kt, 128] so each m-tile is one
    contiguous DMA and [:, :, kc, :] is a [128, 128] lhsT block."""
    return np.ascontiguousarray(
        wT.reshape(kt, 128, mt, 128).transpose(2, 1, 0, 3).astype(dtype))


_CACHE = {}


def kernel(x, ln1_w, ln2_w, qkv_w, proj_w, mlp_w1, mlp_w2):
    x = np.asarray(x, dtype=np.float32)
    ln1_w = np.asarray(ln1_w, dtype=np.float32)
    ln2_w = np.asarray(ln2_w, dtype=np.float32)
    qkv_w = np.asarray(qkv_w, dtype=np.float32)
    proj_w = np.asarray(proj_w, dtype=np.float32)
    mlp_w1 = np.asarray(mlp_w1, dtype=np.float32)
    mlp_w2 = np.asarray(mlp_w2, dtype=np.float32)

    if "nc" not in _CACHE:
        _CACHE["nc"] = build()
    nc = _CACHE["nc"]

    # Fold the LN scales into the consuming weight matrices.
    wqkv = qkv_w * ln1_w[None, :]
    wqk_h = _tile_lhsT(np.ascontiguousarray(wqkv[:2 * DIM].T), DT, 16)
    wv_h = np.ascontiguousarray(wqkv[2 * DIM:].T).astype(
        np.float16).reshape(DT, 128, DIM)
    nws_h = np.ascontiguousarray(
        (-wqkv[:2 * DIM].sum(axis=1)).reshape(1, 16, 128)).astype(np.float16)
    wvs_h = np.ascontiguousarray(
        wqkv[2 * DIM:].sum(axis=1).reshape(1, 2, TOK)).astype(np.float16)
    wproj_h = _tile_lhsT(np.ascontiguousarray(proj_w.T), DT, DT)
    w1_h = _tile_lhsT(np.ascontiguousarray((mlp_w1 * ln2_w[None, :]).T), DT, HT)
    w2_h = _tile_lhsT(np.ascontiguousarray(mlp_w2.T), HT, DT)

    xs = x.reshape(B, RANKS, TOK, DIM)
    in_maps = []
    for c in range(N_CORES):
        b, j = divmod(c, RANKS)
        xTc = np.ascontiguousarray(xs[b, j].T)
        in_maps.append({
            "xh": xTc.astype(np.float16), "xT": xTc,
            "wqk": wqk_h, "wv": wv_h, "nws": nws_h, "wvs": wvs_h,
            "wproj": wproj_h, "w1": w1_h, "w2": w2_h,
        })

    res = bass_utils.run_bass_kernel_spmd(nc, in_maps,
                                          core_ids=list(range(N_CORES)))
    _CACHE["last_results"] = res

    out = np.empty((B, L, DIM), dtype=np.float32)
    for c in range(N_CORES):
        b, j = divmod(c, RANKS)
        out[b, j * TOK:(j + 1) * TOK, :] = res.results[c]["yT"].T
    return out


# revision 44
# speedup vs baseline: 1.2128x; 1.2128x over previous
"""Trainium2 Bass kernel for a dense transformer block.

Strategy: data-parallel over tokens (8 shards of 512 tokens; 4 shards
per batch element, one per core).  Each core computes the full block
for its own tokens.  Attention needs K/V for the whole 2048-token
sequence of the core's batch element.

v2 changes vs the 505us baseline (trace-driven):
  - The K/V exchange is split into EIGHT per-head-pair AllGathers
    (0.25 MiB each) triggered as soon as that pair's K and V tiles are
    stored, so attention pair p starts as soon as gather p lands
    (~40us) instead of waiting for two serialized 50us collectives
    (165us).
  - LayerNorm1 never blocks the PE: Q/K/V matmuls run on RAW x; the
    mean correction is folded in as one extra K=1 matmul per psum
    chain (psum -= colsum(W) x mean), and the 1/std scale is folded
    into the psum->SBUF copies (DVE mul for Q/K, per-partition
    activation scale for V).
  - reciprocal_approx_fast replaces the 3.3us 1-lane reciprocals;
    LN broadcasts ride the PE (ones-matmul) instead of gpsimd.
  - Attention normalize multiplies straight out of PSUM (no oe copy).

Layout: activations live in SBUF as [feature(partition), token(free)]
tiles; weights are pre-transposed/pre-tiled on the host so every
matmul is lhsT[K=128, M=128] x rhs[K=128, N=512].  The residual spine
is fp32; everything wide (Q/K/V, softmax probabilities, weights, MLP
hidden) is fp16.  V is produced directly in [token, feature] layout
(x as the stationary matmul operand) so P@V needs no transposes, and
a ones-column appended to V yields the softmax denominators in the
same accumulation (M=65).
"""

import contextlib

import numpy as np

import concourse.bass as bass  # noqa: F401
import concourse.mybir as mybir
import concourse.tile as tile
from concourse import bacc
from concourse import bass_utils

F32 = mybir.dt.float32
F32R = mybir.dt.float32r
F16 = mybir.dt.float16
F8 = mybir.dt.float8e4
I32 = mybir.dt.int32
AF = mybir.ActivationFunctionType
import numpy as _np
EXP_A = 0.125 * float(_np.log2(_np.e)) * (1 << 23)
EXP_B = float((127 << 23) - 486411)

DIM = 1024
HEADS = 16
HD = 64
HIDDEN = 4096
B = 2
L = 2048
N_CORES = 8
TOK = 512           # tokens per core
DT = DIM // 128     # 8 feature tiles
HT = HIDDEN // 128  # 32 hidden tiles
NPAIR = HEADS // 2  # 8 head pairs (128 features each)
RANKS = 4           # cores per batch group
GROUPS = [[0, 1, 2, 3], [4, 5, 6, 7]]


def _r(ap):
    return ap.bitcast(F32R)


def build():
    nc = bacc.Bacc("TRN2", target_bir_lowering=False, debug=False,
                   num_devices=N_CORES)

    xh = nc.dram_tensor("xh", [DIM, TOK], F16, kind="ExternalInput").ap()
    xT = nc.dram_tensor("xT", [DIM, TOK], F32, kind="ExternalInput").ap()
    # lhsT-tiled weights: [m_tiles, 128(k_inner), k_tiles, 128(m_inner)]
    wqk = nc.dram_tensor("wqk", [16, 128, DT, 128], F16, kind="ExternalInput").ap()
    wv = nc.dram_tensor("wv", [DT, 128, DIM], F16, kind="ExternalInput").ap()
    nws = nc.dram_tensor("nws", [1, 16, 128], F16, kind="ExternalInput").ap()
    wvs = nc.dram_tensor("wvs", [1, 2, TOK], F16, kind="ExternalInput").ap()
    wproj = nc.dram_tensor("wproj", [DT, 128, DT, 128], F16, kind="ExternalInput").ap()
    w1 = nc.dram_tensor("w1", [HT, 128, DT, 128], F16, kind="ExternalInput").ap()
    w2 = nc.dram_tensor("w2", [DT, 128, HT, 128], F16, kind="ExternalInput").ap()
    yT = nc.dram_tensor("yT", [DIM, TOK], F32, kind="ExternalOutput").ap()

    with tile.TileContext(nc) as tc:
        with contextlib.ExitStack() as ctx:
            # ---- long-lived pools -------------------------------------
            const = ctx.enter_context(tc.tile_pool(name="const", bufs=1))
            rows = ctx.enter_context(tc.tile_pool(name="rows", bufs=3))
            rows1 = ctx.enter_context(tc.tile_pool(name="rows1", bufs=1))
            xhp = ctx.enter_context(tc.tile_pool(name="xhp", bufs=8))
            x32p = ctx.enter_context(tc.tile_pool(name="x32p", bufs=2))
            qp_pool = ctx.enter_context(tc.tile_pool(name="qp", bufs=8))
            atp = ctx.enter_context(tc.tile_pool(name="atp", bufs=8))
            Xp = ctx.enter_context(tc.tile_pool(name="Xp", bufs=8))
            ynorm = ctx.enter_context(tc.tile_pool(name="ynorm", bufs=8))
            yop = ctx.enter_context(tc.tile_pool(name="yop", bufs=1))
            dram = ctx.enter_context(tc.tile_pool(name="dram", bufs=1, space="DRAM"))
            # attention SBUF pools; allocate (pin) every ring buffer NOW,
            # before the QKV-phase pools claim space, so the attention
            # tiles never alias QKV tiles -> no false serialization at
            # the QKV->attention handoff via address reuse.
            kp_pool = ctx.enter_context(tc.tile_pool(name="kp", bufs=6))
            kp8_pool = ctx.enter_context(tc.tile_pool(name="kp8", bufs=4))
            va_pool = ctx.enter_context(tc.tile_pool(name="va", bufs=6))
            va8_pool = ctx.enter_context(tc.tile_pool(name="va8", bufs=6))
            ex_pool = ctx.enter_context(tc.tile_pool(name="ex", bufs=4))
            bcr_p = ctx.enter_context(tc.tile_pool(name="bcr", bufs=2))
            for i in range(6):
                t = kp_pool.tile([128, TOK], F16, tag="kp", name=f"kpd{i}")
                nc.vector.memset(t[0:1, 0:1], 0.0)
                if i < 4:
                    t = kp8_pool.tile([128, TOK], F8, tag="kp8", name=f"kp8d{i}")
                    nc.vector.memset(t[0:1, 0:1], 0.0)
            for i in range(6):
                t = va_pool.tile([128, 4, 2, HD + 1], F16, tag="va",
                                 name=f"vad{i}")
                nc.vector.memset(t[0:1, 0:1, 0:1, 0:1], 0.0)
                t = va8_pool.tile([128, 4, 2, HD], F8, tag="va8",
                                  name=f"va8d{i}")
                nc.vector.memset(t[0:1, 0:1, 0:1, 0:1], 0.0)
            for i in range(4):
                t = ex_pool.tile([128, 2, TOK], F16, tag="ex", name=f"exd{i}")
                nc.vector.memset(t[0:1, 0:1, 0:1], 0.0)
            for i in range(2):
                t = bcr_p.tile([HD, TOK], F32, tag="bcr", name=f"bcrd{i}")
                nc.vector.memset(t[0:1, 0:1], 0.0)


            ones16 = const.tile([128, 1], F16)
            nc.vector.memset(ones16[:], 1.0)
            ones32 = const.tile([128, 1], F32)
            nc.vector.memset(ones32[:], 1.0)
            onesrow = const.tile([1, 128], F32)
            nc.vector.memset(onesrow[:], 1.0)
            nws_sb = const.tile([1, 16, 128], F16)
            nc.sync.dma_start(out=nws_sb[:], in_=nws)
            wvs_sb = const.tile([1, 2, TOK], F16)
            nc.sync.dma_start(out=wvs_sb[:], in_=wvs)
            r_col4 = const.tile([128, 4], F32)

            warm_in = dram.tile([1, 16], F16)
            warm_out = dram.tile([RANKS, 1, 16], F16)
            kv_in = [dram.tile([2, 128, TOK], F8, name=f"kvi{p}")
                     for p in range(NPAIR)]
            kv_out = [dram.tile([RANKS, 2, 128, TOK], F8, name=f"kvo{p}")
                      for p in range(NPAIR)]

            # Warm up the collective subsystem (init barrier + ncfw)
            # immediately so the real gathers below do not pay the
            # first-collective penalty.
            wz = rows.tile([1, 16], F16, tag="wz")
            nc.vector.memset(wz[:], 0.0)
            nc.sync.dma_start(out=warm_in[0], in_=wz[:])
            nc.gpsimd.collective_compute(
                "AllGather", mybir.AluOpType.bypass,
                replica_groups=GROUPS,
                ins=[warm_in.opt()], outs=[warm_out.opt()])

            # ---- load x (fp16 for matmuls; fp32 later for residual) ----
            xh_tiles = []
            for dc in range(DT):
                t = xhp.tile([128, TOK], F16, tag="xh")
                nc.sync.dma_start(out=t[:], in_=xh[dc * 128:(dc + 1) * 128, :])
                xh_tiles.append(t)

            with contextlib.ExitStack() as actx:
                sqp = actx.enter_context(tc.tile_pool(name="sqp", bufs=2))
                wqk_pool = actx.enter_context(tc.tile_pool(name="wqkp", bufs=16))
                wv_pool = actx.enter_context(tc.tile_pool(name="wvp", bufs=2))
                kvtmp = actx.enter_context(tc.tile_pool(name="kvtmp", bufs=4))
                psQ = actx.enter_context(tc.tile_pool(name="psQ", bufs=2, space="PSUM"))
                lnps = actx.enter_context(tc.tile_pool(name="lnps", bufs=2, space="PSUM"))
                rbcps = actx.enter_context(tc.tile_pool(name="rbcps", bufs=1, space="PSUM"))
                ps1 = actx.enter_context(tc.tile_pool(name="ps1", bufs=2, space="PSUM"))

                # ---- LN1 stats on fp16 raw x (rides along) ------------
                sum_ps = lnps.tile([1, TOK], F32, tag="lnps", name="sum")[:]
                sq_ps = lnps.tile([1, TOK], F32, tag="lnps", name="sq")[:]
                sq_tiles = []
                for dc in range(DT):
                    sq = sqp.tile([128, TOK], F16, tag="sq")
                    nc.vector.tensor_mul(sq[:], xh_tiles[dc][:], xh_tiles[dc][:])
                    sq_tiles.append(sq)
                for dc in range(DT):
                    nc.tensor.matmul(sum_ps, ones16[:], xh_tiles[dc][:],
                                     start=(dc == 0), stop=(dc == DT - 1),
                                     skip_group_check=True)
                for dc in range(DT):
                    nc.tensor.matmul(sq_ps, ones16[:], sq_tiles[dc][:],
                                     start=(dc == 0), stop=(dc == DT - 1),
                                     skip_group_check=True)

                m16 = rows.tile([1, TOK], F16, tag="r16")
                negm16 = rows.tile([1, TOK], F16, tag="r16")
                m32 = rows.tile([1, TOK], F32, tag="r32")
                ex2 = rows.tile([1, TOK], F32, tag="r32")
                var = rows.tile([1, TOK], F32, tag="r32")
                std = rows.tile([1, TOK], F32, tag="r32")
                r32 = rows.tile([1, TOK], F32, tag="r32")
                nc.vector.tensor_scalar_mul(m16[:], sum_ps, 1.0 / DIM)
                nc.vector.tensor_scalar_mul(negm16[:], sum_ps, -1.0 / DIM)
                nc.vector.tensor_scalar_mul(m32[:], sum_ps, 1.0 / DIM)
                nc.vector.tensor_scalar_mul(ex2[:], sq_ps, 1.0 / DIM)
                nc.vector.tensor_mul(var[:], m32[:], m32[:])
                nc.vector.tensor_sub(var[:], ex2[:], var[:])
                nc.scalar.activation(std[:], var[:], AF.Sqrt)
                nc.vector.reciprocal_approx_fast(r32[:], std[:])
                # broadcast r over partitions via the PE; column-chunked r
                # for the V-side per-partition scale via a tiny DMA gather
                rbc = rbcps.tile([128, TOK], F32, tag="rbc")
                nc.tensor.matmul(rbc[:], onesrow[:], r32[:],
                                 start=True, stop=True)
                rbc_sb = xhp.tile([128, TOK], F32, tag="rbcsb", bufs=1)
                nc.vector.tensor_copy(rbc_sb[:], rbc[:])
                r_scr = dram.tile([1, TOK], F32, name="rscr")
                nc.sync.dma_start(out=r_scr[:], in_=r32[:])
                nc.sync.dma_start(
                    out=r_col4[:],
                    in_=r_scr[0:1, :].rearrange("o (c p) -> (o p) c", p=128))

                def qk_chain(et, dest_pool, dest_tag, dest_name,
                             dest_dtype=F16):
                    """One Q or K etile: 8 raw MMs + mean-correction MM,
                    then scale-by-r copy to SBUF."""
                    wt = wqk_tiles[et]
                    ps_pool = psQ if et < 8 else ps1
                    ps = ps_pool.tile([128, TOK], F32, tag="mm", name=f"qk{et}")
                    for dc in range(DT):
                        nc.tensor.matmul(ps[:], wt[:, dc, :],
                                         xh_tiles[dc][:],
                                         start=(dc == 0), stop=False)
                    nc.tensor.matmul(ps[:], nws_sb[:, et, :], m16[:],
                                     start=False, stop=True)
                    dest = dest_pool.tile([128, TOK], dest_dtype, tag=dest_tag,
                                          name=dest_name)
                    nc.vector.tensor_mul(dest[:], ps[:], rbc_sb[:])
                    return dest

                # contiguous per-etile weight loads; K etiles (8..15) first
                wqk_tiles = [None] * 16
                for et in list(range(8, 16)) + list(range(8)):
                    wt = wqk_pool.tile([128, DT, 128], F16, tag="wqk",
                                       name=f"wqk{et}")
                    nc.sync.dma_start(out=wt[:], in_=wqk[et])
                    wqk_tiles[et] = wt

                def v_block(nh):
                    """V for feature half nh: 4 token-chunk chains, each
                    [128 tok, 512 feat]; store per-pair slices."""
                    wvt = wv_pool.tile([128, DT, TOK], F16, tag="wv",
                                       name=f"wv{nh}")
                    nc.sync.dma_start(
                        out=wvt[:],
                        in_=wv[:, :, nh * TOK:(nh + 1) * TOK].rearrange(
                            "d k f -> k d f"))
                    for tt in range(4):
                        ps = ps1.tile([128, TOK], F32, tag="mm",
                                      name=f"v{nh}_{tt}")
                        for dc in range(DT):
                            nc.tensor.matmul(
                                ps[:], xh_tiles[dc][:, tt * 128:(tt + 1) * 128],
                                wvt[:, dc, :],
                                start=(dc == 0), stop=False)
                        nc.tensor.matmul(
                            ps[:], negm16[:, tt * 128:(tt + 1) * 128],
                            wvs_sb[:, nh, :], start=False, stop=True)
                        vt = kvtmp.tile([128, TOK], F8, tag="kv",
                                        name=f"vt{nh}_{tt}")
                        nc.scalar.activation(vt[:], ps[:], AF.Copy,
                                             scale=r_col4[:, tt:tt + 1])
                        for j in range(4):
                            p = nh * 4 + j
                            nc.scalar.dma_start(
                                out=kv_in[p][1, :, tt * 128:(tt + 1) * 128],
                                in_=vt[:, j * 128:(j + 1) * 128])

                def k_chain(p):
                    kt = qk_chain(8 + p, kvtmp, "kv", f"kt{p}", F8)
                    nc.scalar.dma_start(out=kv_in[p][0], in_=kt[:])

                def gather(p):
                    nc.gpsimd.collective_compute(
                        "AllGather", mybir.AluOpType.bypass,
                        replica_groups=GROUPS,
                        ins=[kv_in[p].opt()], outs=[kv_out[p].opt()])

                q_tiles = [None] * NPAIR

                for p in range(4):
                    k_chain(p)
                v_block(0)
                for p in range(4):
                    gather(p)
                for p in range(NPAIR):
                    q_tiles[p] = qk_chain(p, qp_pool, "q", f"q{p}")
                for p in range(4, 8):
                    k_chain(p)
                v_block(1)
                for p in range(4, 8):
                    gather(p)

            # ---- attention + weight prefetch --------------------------
            attn_tiles = []
            wp_tiles = []
            w1_tiles = []
            with tc.tile_pool(name="wproj", bufs=2) as wp_pool, \
                 tc.tile_pool(name="w1", bufs=4) as w1_pool, \
                 contextlib.ExitStack() as attps:
                pss = attps.enter_context(
                    tc.tile_pool(name="pss", bufs=2, space="PSUM"))
                pso = attps.enter_context(
                    tc.tile_pool(name="pso", bufs=4, space="PSUM"))

                def load_w1(g):
                    wt = w1_pool.tile([128, 4, DT, 128], F16, tag="w1",
                                      name=f"w1_{g}")
                    nc.sync.dma_start(
                        out=wt[:],
                        in_=w1[g * 4:(g + 1) * 4].rearrange("e k d m -> k e d m"))
                    w1_tiles.append(wt)

                def prefetch(p):
                    # trickle the proj/fc1 weights in between pairs; only
                    # into FRESH ring slots (a recycled slot's DMA would
                    # park in the sync queue waiting on MLP-time readers
                    # and stall the attention loads queued behind it)
                    if p < 2:
                        wt = wp_pool.tile([128, 4, DT, 128], F16, tag="wp",
                                          name=f"wp{p}")
                        nc.sync.dma_start(
                            out=wt[:],
                            in_=wproj[p * 4:(p + 1) * 4].rearrange(
                                "e k d m -> k e d m"))
                        wp_tiles.append(wt)
                    if p < 4:
                        load_w1(p)

                for p in range(NPAIR):
                    qp = q_tiles[p]
                    kps, vas = [], []
                    for r_i in range(RANKS):
                        kp8 = kp8_pool.tile([128, TOK], F8, tag="kp8",
                                            name=f"kp8{p}_{r_i}")
                        nc.sync.dma_start(out=kp8[:],
                                          in_=kv_out[p][r_i, 0])
                        kp = kp_pool.tile([128, TOK], F16, tag="kp",
                                          name=f"kp{p}_{r_i}")
                        nc.vector.tensor_copy(kp[:], kp8[:])
                        kps.append(kp)
                    for r_i in range(RANKS):
                        va8 = va8_pool.tile([128, 4, 2, HD], F8, tag="va8",
                                            name=f"va8{p}_{r_i}")
                        nc.sync.dma_start(
                            out=va8[:],
                            in_=kv_out[p][r_i, 1].rearrange(
                                "t (c h d) -> t c h d", c=4, d=HD))
                        va = va_pool.tile([128, 4, 2, HD + 1], F16,
                                          tag="va", name=f"va{p}_{r_i}")
                        nc.gpsimd.tensor_copy(va[:, :, :, 0:HD], va8[:])
                        nc.vector.memset(va[:, :, :, HD:HD + 1], 1.0)
                        vas.append(va)
                    o0 = pso.tile([HD + 1, TOK], F32, tag="pso",
                                  name=f"o0_{p}")
                    o1 = pso.tile([HD + 1, TOK], F32, tag="pso",
                                  name=f"o1_{p}")
                    for kt in range(16):
                        r_i, cc = kt // 4, kt % 4
                        kp, va = kps[r_i], vas[r_i]
                        ss = pss.tile([128, 2, TOK], F32, tag="pss")
                        ex = ex_pool.tile([128, 2, TOK], F16, tag="ex")
                        nc.tensor.matmul(
                            ss[:, 0, :],
                            kp[0:HD, cc * 128:(cc + 1) * 128],
                            qp[0:HD, :], start=True, stop=True)
                        nc.tensor.matmul(
                            ss[:, 1, :],
                            kp[HD:128, cc * 128:(cc + 1) * 128],
                            qp[HD:128, :], start=True, stop=True)
                        nc.scalar.activation(ex[:], ss[:], AF.Exp,
                                             scale=float(HD) ** -0.5)
                        nc.tensor.matmul(o0[:], va[:, cc, 0, :],
                                         ex[:, 0, :],
                                         start=(kt == 0), stop=(kt == 15))
                        nc.tensor.matmul(o1[:], va[:, cc, 1, :],
                                         ex[:, 1, :],
                                         start=(kt == 0), stop=(kt == 15))
                    at = atp.tile([128, TOK], F16, tag="at", name=f"at{p}")
                    for h_i, o in ((0, o0), (1, o1)):
                        # reciprocal_approx_* mishandles base_partition!=0
                        # inputs -> bounce the denominator row to part 0
                        den = rows1.tile([1, TOK], F32, tag="den")
                        nc.vector.tensor_copy(den[:], o[HD:HD + 1, :])
                        rc = rows.tile([1, TOK], F32, tag="rc")
                        nc.vector.reciprocal_approx_fast(rc[:], den[:])
                        bcr = bcr_p.tile([HD, TOK], F32, tag="bcr")
                        nc.gpsimd.partition_broadcast(bcr[:], rc[:])
                        nc.vector.tensor_mul(at[h_i * HD:(h_i + 1) * HD, :],
                                             o[0:HD, :], bcr[:])
                    attn_tiles.append(at)
                    prefetch(p)

                attps.close()
                # ---- proj + residual, LN2 (stats interleaved) ---------
                X_tiles = []
                with tc.tile_pool(name="tmp", bufs=4) as tmp, \
                     tc.tile_pool(name="ps3", bufs=4, space="PSUM") as ps3, \
                     tc.tile_pool(name="lnps2", bufs=2, space="PSUM") as lnps2, \
                     tc.tile_pool(name="bc2", bufs=2, space="PSUM") as bc2:
                    sum2 = lnps2.tile([1, TOK], F32, tag="lnps")
                    sq2 = lnps2.tile([1, TOK], F32, tag="lnps")
                    x_tiles = []
                    for dc in range(DT):
                        t = x32p.tile([128, TOK], F32, tag="x",
                                      name=f"x{dc}")
                        nc.scalar.dma_start(
                            out=_r(t[:]),
                            in_=_r(xT[dc * 128:(dc + 1) * 128, :]))
                        x_tiles.append(t)
                    for et in range(DT):
                        wt = wp_tiles[et // 4]
                        ps = ps3.tile([128, TOK], F32, tag="mm")
                        for dc in range(DT):
                            nc.tensor.matmul(ps[:], wt[:, et % 4, dc, :],
                                             attn_tiles[dc][:],
                                             start=(dc == 0),
                                             stop=(dc == DT - 1))
                        xt = Xp.tile([128, TOK], F32, tag="X",
                                     name=f"X{et}")
                        nc.vector.tensor_add(_r(xt[:]), ps[:],
                                             x_tiles[et][:])
                        X_tiles.append(xt)
                        sq = tmp.tile([128, TOK], F32, tag="lntmp", bufs=3)
                        nc.vector.tensor_mul(_r(sq[:]), xt[:], xt[:])
                        nc.tensor.matmul(sum2[:], _r(ones32[:]), _r(xt[:]),
                                         start=(et == 0), stop=(et == DT - 1))
                        nc.tensor.matmul(sq2[:], _r(ones32[:]), _r(sq[:]),
                                         start=(et == 0), stop=(et == DT - 1))

                    m2 = rows.tile([1, TOK], F32, tag="r32b")
                    e2 = rows.tile([1, TOK], F32, tag="r32b")
                    v2 = rows.tile([1, TOK], F32, tag="r32b")
                    s2 = rows.tile([1, TOK], F32, tag="r32b")
                    r2 = rows.tile([1, TOK], F32, tag="r32b")
                    nc.vector.tensor_scalar_mul(m2[:], sum2[:], 1.0 / DIM)
                    mbc = bc2.tile([128, TOK], F32, tag="bc")
                    nc.tensor.matmul(mbc[:], onesrow[:], m2[:],
                                     start=True, stop=True)
                    nc.vector.tensor_scalar_mul(e2[:], sq2[:], 1.0 / DIM)
                    nc.vector.tensor_mul(v2[:], m2[:], m2[:])
                    nc.vector.tensor_sub(v2[:], e2[:], v2[:])
                    nc.scalar.activation(s2[:], v2[:], AF.Sqrt)
                    nc.vector.reciprocal_approx_fast(r2[:], s2[:])
                    rbc2 = bc2.tile([128, TOK], F32, tag="bc")
                    nc.tensor.matmul(rbc2[:], onesrow[:], r2[:],
                                     start=True, stop=True)
                    Y_tiles = []
                    for dc in range(DT):
                        tm = tmp.tile([128, TOK], F32, tag="lnap")
                        nc.vector.tensor_sub(tm[:], X_tiles[dc][:], mbc[:])
                        y = ynorm.tile([128, TOK], F16, tag="y",
                                       name=f"y{dc}")
                        nc.vector.tensor_mul(y[:], tm[:], rbc2[:])
                        Y_tiles.append(y)

                # ---- fc1 + gelu, fc2 + residual -----------------------
                with tc.tile_pool(name="hp", bufs=32) as hp, \
                     tc.tile_pool(name="ps4", bufs=4, space="PSUM") as ps4:
                    for g in range(4, 8):
                        load_w1(g)
                    h_tiles = []
                    for ht in range(HT):
                        wt = w1_tiles[ht // 4]
                        ps = ps4.tile([128, TOK], F32, tag="mm")
                        for dc in range(DT):
                            nc.tensor.matmul(ps[:], wt[:, ht % 4, dc, :],
                                             Y_tiles[dc][:],
                                             start=(dc == 0),
                                             stop=(dc == DT - 1))
                        h = hp.tile([128, TOK], F16, tag="h")
                        nc.scalar.activation(h[:], ps[:], AF.Gelu)
                        h_tiles.append(h)

                    with tc.tile_pool(name="w2", bufs=2) as w2_pool:
                        for et in range(DT):
                            wts = []
                            for hh in range(2):
                                wt = w2_pool.tile([128, HT // 2, 128], F16,
                                                  tag="w2")
                                nc.sync.dma_start(
                                    out=wt[:],
                                    in_=w2[et][:, hh * 16:(hh + 1) * 16, :])
                                wts.append(wt)
                            ps = ps4.tile([128, TOK], F32, tag="mm")
                            for hc in range(HT):
                                nc.tensor.matmul(ps[:], wts[hc // 16][:, hc % 16, :],
                                                 h_tiles[hc][:],
                                                 start=(hc == 0),
                                                 stop=(hc == HT - 1))
                            ot = yop.tile([128, TOK], F32, tag="yo")
                            nc.vector.tensor_add(ot[:], ps[:],
                                                 X_tiles[et][:])
                            nc.sync.dma_start(
                                out=yT[et * 128:(et + 1) * 128, :],
                                in_=ot[:])

    nc.compile()
    return nc


def _tile_lhsT(wT, kt, mt, dtype=np.float16):
    """[Ktot, Mtot] -> [mt, 128, kt, 128] so each m-tile is one
    contiguous DMA and [:, :, kc, :] is a [128, 128] lhsT block."""
    return np.ascontiguousarray(
        wT.reshape(kt, 128, mt, 128).transpose(2, 1, 0, 3).astype(dtype))


_CACHE = {}


def kernel(x, ln1_w, ln2_w, qkv_w, proj_w, mlp_w1, mlp_w2):
    x = np.asarray(x, dtype=np.float32)
    ln1_w = np.asarray(ln1_w, dtype=np.float32)
    ln2_w = np.asarray(ln2_w, dtype=np.float32)
    qkv_w = np.asarray(qkv_w, dtype=np.float32)
    proj_w = np.asarray(proj_w, dtype=np.float32)
    mlp_w1 = np.asarray(mlp_w1, dtype=np.float32)
    mlp_w2 = np.asarray(mlp_w2, dtype=np.float32)

    if "nc" not in _CACHE:
        _CACHE["nc"] = build()
    nc = _CACHE["nc"]

    # Fold the LN scales into the consuming weight matrices.
    wqkv = qkv_w * ln1_w[None, :]
    wqk_h = _tile_lhsT(np.ascontiguousarray(wqkv[:2 * DIM].T), DT, 16)
    wv_h = np.ascontiguousarray(wqkv[2 * DIM:].T).astype(
        np.float16).reshape(DT, 128, DIM)
    nws_h = np.ascontiguousarray(
        (-wqkv[:2 * DIM].sum(axis=1)).reshape(1, 16, 128)).astype(np.float16)
    wvs_h = np.ascontiguousarray(
        wqkv[2 * DIM:].sum(axis=1).reshape(1, 2, TOK)).astype(np.float16)
    wproj_h = _tile_lhsT(np.ascontiguousarray(proj_w.T), DT, DT)
    w1_h = _tile_lhsT(np.ascontiguousarray((mlp_w1 * ln2_w[None, :]).T), DT, HT)
    w2_h = _tile_lhsT(np.ascontiguousarray(mlp_w2.T), HT, DT)

    xs = x.reshape(B, RANKS, TOK, DIM)
    in_maps = []
    for c in range(N_CORES):
        b, j = divmod(c, RANKS)
        xTc = np.ascontiguousarray(xs[b, j].T)
        in_maps.append({
            "xh": xTc.astype(np.float16), "xT": xTc,
            "wqk": wqk_h, "wv": wv_h, "nws": nws_h, "wvs": wvs_h,
            "wproj": wproj_h, "w1": w1_h, "w2": w2_h,
        })

    res = bass_utils.run_bass_kernel_spmd(nc, in_maps,
                                          core_ids=list(range(N_CORES)))
    _CACHE["last_results"] = res

    out = np.empty((B, L, DIM), dtype=np.float32)
    for c in range(N_CORES):
        b, j = divmod(c, RANKS)
        out[b, j * TOK:(j + 1) * TOK, :] = res.results[c]["yT"].T
    return out
